# revision 22
# baseline (speedup 1.0000x reference)
"""SchNet-style GNN message passing on 8 Trainium2 NeuronCores.

Strategy: edges sharded by destination atom across 8 cores; atoms packed
into 128-atom bins balanced on in-degree, bins dealt to (core, chunk)
slots so every core runs an identical static schedule.  Small weights
replicated.  Per conv: each core computes the atom-filter features (hf)
for its own shard only and one AllGather builds the full gather table in
DRAM; per-edge source features are fetched with hardware gather-DMA
(trailing pad indices are -1, which the Q7 descriptor generator trims for
free); the scatter-add is one-hot matmuls into PSUM accumulators.  The
ssp -log(2) shift is folded into the Ln activation (Ln(0.5x+0.5)), which
removes all bias matmuls (layer-2 biases are zero in this model).  The
readout runs per-chunk fused into the last conv's update, accumulating
per-molecule energies in a single PSUM group; host sums the 8 partials.
"""

import os
import sys
import numpy as np

sys.path.insert(0, "/opt/trn_rl_repo")

from contextlib import ExitStack

import ml_dtypes
import concourse.bass as bass
import concourse.tile as tile
import concourse.bacc as bacc
from concourse import mybir
from concourse import bass_utils

F32 = mybir.dt.float32
BF16 = mybir.dt.bfloat16
I16 = mybir.dt.int16
AF = mybir.ActivationFunctionType
OP = mybir.AluOpType

LN2 = float(np.log(2.0))
EPS = 1e-12
P = 128          # partitions / chunk size
NG = 32          # gaussians
NB = 128         # atom basis / filters
NH = 64          # readout hidden


# ----------------------------------------------------------------------------
# Host-side plan: atom binning, edge sharding, static schedule
# ----------------------------------------------------------------------------

class Plan:
    pass


def _greedy_pack(deg, atom_ids, n_bins):
    """Pack len(atom_ids) atoms into n_bins bins of exactly P atoms each,
    balancing per-bin sums of deg.  Returns [n_bins, P] atom ids."""
    n = len(atom_ids)
    assert n == n_bins * P
    order = np.argsort(-deg[atom_ids], kind="stable")
    s = np.zeros(n_bins)
    cnt = np.zeros(n_bins, dtype=np.int64)
    bins = np.full((n_bins, P), -1, dtype=np.int64)
    for oi in order:
        a = atom_ids[oi]
        load = s + deg[a]
        load[cnt >= P] = np.inf
        i = int(np.argmin(load))
        bins[i, cnt[i]] = a
        cnt[i] += 1
        s[i] += deg[a]
    assert (cnt == P).all()
    return bins


def make_plan(r, xyz, a, n_per, n_cores=8):
    pl = Plan()
    n_atoms = xyz.shape[0]
    n_edges = a.shape[0]
    rng = np.random.default_rng(12345)

    npad = ((n_atoms + n_cores * P - 1) // (n_cores * P)) * (n_cores * P)
    K = npad // (n_cores * P)          # chunks (slots) per core
    SH = K * P                          # atoms per core shard
    a_cap = min(32768, npad)            # rows addressable from table base A
    a_cap = (a_cap // P) * P
    b_base = npad - a_cap               # base row of table B
    assert npad - b_base <= 32768

    dst = a[:, 0].astype(np.int64)
    src = a[:, 1].astype(np.int64)
    n_virt = npad - n_atoms

    deg = np.bincount(dst, minlength=n_atoms).astype(np.float64)
    deg_x = np.concatenate([deg, np.zeros(n_virt)])

    # bins balanced on total in-degree, globally; dealt to (core, slot)
    all_ids = rng.permutation(npad)
    bins = _greedy_pack(deg_x, all_ids, npad // P)      # [npad//P, P]

    # deal bins to (core, slot): two passes — the A/B split of a bin's
    # in-edges depends on where SOURCE bins land, so deal once, measure each
    # bin's A-count under that layout, then re-deal with slot c taking the
    # c-th run of 8 bins sorted by A-count (cntB anti-correlates, so both
    # halves end up tight per slot)
    n_bins = npad // P
    assert n_bins == K * n_cores

    def apply_deal(bin_order):
        bin_of = bin_order.reshape(K, n_cores)
        new_of_old = np.full(npad, -1, dtype=np.int64)
        old_of_new = np.empty(npad, dtype=np.int64)
        for m in range(n_cores):
            for c in range(K):
                rows = bins[bin_of[c, m]]
                base = m * SH + c * P
                old_of_new[base:base + P] = rows
                new_of_old[rows] = np.arange(base, base + P)
        return bin_of, new_of_old, old_of_new

    def half_counts(new_of_old):
        src_new = new_of_old[src]
        dst_new = new_of_old[dst]
        e_core = dst_new // SH
        e_chunk = (dst_new % SH) // P
        e_half = (src_new >= a_cap).astype(np.int64)
        cnt = np.zeros((n_cores, K, 2), dtype=np.int64)
        np.add.at(cnt, (e_core, e_chunk, e_half), 1)
        return src_new, dst_new, e_core, e_chunk, e_half, cnt

    order0 = np.arange(n_bins)
    _, new0, _ = apply_deal(order0)
    # per-bin A-count under the provisional layout
    sN, dN, eC, eK2, eH, cnt0 = half_counts(new0)
    binA = np.zeros(n_bins)
    dst_bin0 = np.empty(npad, dtype=np.int64)
    bin_of0 = order0.reshape(K, n_cores)
    for m in range(n_cores):
        for c in range(K):
            dst_bin0[bins[bin_of0[c, m]]] = bin_of0[c, m]
    np.add.at(binA, dst_bin0[dst], 1 - eH)
    order1 = np.argsort(-binA, kind="stable")
    bin_of, new_of_old, old_of_new = apply_deal(order1)
    src_new, dst_new, e_core, e_chunk, e_half, cnt = half_counts(new_of_old)
    maxc = cnt.max(axis=0)                          # [K, 2]
    # keep whichever deal has the smaller gather total
    cs0 = cnt0.max(axis=0).sum()
    if maxc.sum() > cs0:
        bin_of, new_of_old, old_of_new = apply_deal(order0)
        src_new, dst_new, e_core, e_chunk, e_half, cnt = half_counts(new_of_old)
        maxc = cnt.max(axis=0)
    LA = np.maximum(((maxc[:, 0] + P - 1) // P + 3) // 4 * 4, 4)
    LB = np.maximum(((maxc[:, 1] + P - 1) // P + 3) // 4 * 4, 4)

    # static stream: per chunk, A-span then B-span; pad total to 2048 edges
    spans = []          # (st0, n_subtiles, half, chunk)
    st_chunk = []
    s = 0
    for c in range(K):
        spans.append((s, int(LA[c]), 0, c))
        st_chunk += [c] * int(LA[c])
        s += int(LA[c])
        spans.append((s, int(LB[c]), 1, c))
        st_chunk += [c] * int(LB[c])
        s += int(LB[c])
    padb = (16 - (s % 16)) % 16
    if padb:
        st0, ns, hh, cc = spans[-1]
        spans[-1] = (st0, ns + padb, hh, cc)
        st_chunk += [K - 1] * padb
        s += padb
    n_sub = s
    Ep = n_sub * P
    st_chunk = np.asarray(st_chunk, dtype=np.int64)
    assert all(ns <= 32 for _, ns, _, _ in spans), max(ns for _, ns, _, _ in spans)

    chunk_first = np.zeros(K, dtype=np.int64)
    chunk_last = np.zeros(K, dtype=np.int64)
    seen = set()
    for st in range(n_sub):
        c = st_chunk[st]
        if c not in seen:
            chunk_first[c] = st
            seen.add(c)
        chunk_last[c] = st

    span_start = {}
    for st0, ns, hh, cc in spans:
        span_start[(cc, hh)] = st0

    # ---- per-core edge data -------------------------------------------------
    idx_lin = np.full((n_cores, Ep), -1, dtype=np.int16)
    dstrel_lin = np.full((n_cores, Ep), -1.0, dtype=np.float32)
    osrc_lin = np.zeros((n_cores, Ep), dtype=np.int64)
    odst_lin = np.zeros((n_cores, Ep), dtype=np.int64)

    order = np.lexsort((e_half, e_chunk, e_core))
    so_src, so_dst = src_new[order], dst_new[order]
    so_osrc, so_odst = src[order], dst[order]
    keys = e_core[order] * (K * 2) + e_chunk[order] * 2 + e_half[order]
    bstart = np.searchsorted(keys, np.arange(n_cores * K * 2), side="left")
    bend = np.searchsorted(keys, np.arange(n_cores * K * 2), side="right")

    # per-span gather count (common across cores; the num_idxs register is
    # baked into the SPMD program): fill [cnt, cap) with index 0 so every
    # index before the cap is valid, and only [cap, span_end) stays -1.
    span_cap = {}
    for c in range(K):
        for h in (0, 1):
            span_cap[(c, h)] = max(int(maxc[c, h]), 1)

    for core in range(n_cores):
        for c in range(K):
            for h in (0, 1):
                bi = core * (K * 2) + c * 2 + h
                e0, e1 = bstart[bi], bend[bi]
                ecnt = e1 - e0
                p0 = span_start[(c, h)] * P
                sl = slice(p0, p0 + ecnt)
                if h == 0:
                    idx_lin[core, sl] = so_src[e0:e1].astype(np.int16)
                else:
                    idx_lin[core, sl] = (so_src[e0:e1] - b_base).astype(np.int16)
                dstrel_lin[core, sl] = (so_dst[e0:e1] % P).astype(np.float32)
                osrc_lin[core, sl] = so_osrc[e0:e1]
                odst_lin[core, sl] = so_odst[e0:e1]
                cap = span_cap[(c, h)]
                if ecnt < cap:
                    idx_lin[core, p0 + ecnt:p0 + cap] = 0

    pl.n_atoms, pl.n_edges, pl.npad = n_atoms, n_edges, npad
    pl.n_cores, pl.K, pl.SH, pl.Ep, pl.n_sub = n_cores, K, SH, Ep, n_sub
    pl.a_cap, pl.b_base = a_cap, b_base
    pl.spans = spans
    pl.span_cap = span_cap
    pl.st_chunk, pl.chunk_first, pl.chunk_last = st_chunk, chunk_first, chunk_last
    pl.new_of_old, pl.old_of_new = new_of_old, old_of_new
    pl.idx_lin, pl.dstrel_lin = idx_lin, dstrel_lin
    pl.osrc_lin, pl.odst_lin = osrc_lin, odst_lin
    pl.n_per = int(n_per)
    pl.n_mol = n_atoms // pl.n_per
    return pl


def make_inputs(pl, r, xyz, a, embed, weights):
    C, K, SH, Ep, n_sub = pl.n_cores, pl.K, pl.SH, pl.Ep, pl.n_sub
    NC = weights["fw1"].shape[0]
    NM = pl.n_mol
    F0 = Ep // P

    h0_all = embed[r[:, 0].astype(np.int64)].astype(np.float32)
    h0_new = np.zeros((pl.npad, NB), dtype=np.float32)
    real = pl.old_of_new < pl.n_atoms
    h0_new[real] = h0_all[pl.old_of_new[real]]


    mol_new = np.full(pl.npad, -1, dtype=np.int64)
    mol_new[real] = pl.old_of_new[real] // pl.n_per

    xyzf = xyz.astype(np.float32)

    fw1, fb1 = weights["fw1"], weights["fb1"]
    fw2, fb2 = weights["fw2"], weights["fb2"]
    afw, afb = weights["afw"], weights["afb"]
    ow1, ob1 = weights["ow1"], weights["ob1"]
    ow2, ob2 = weights["ow2"], weights["ob2"]
    aw1, ab1 = weights["aw1"], weights["ab1"]
    aw2, ab2 = weights["aw2"], weights["ab2"]
    # the Ln(0.5x+0.5) ssp fold relies on zero layer-2 biases
    assert np.all(afb == 0.0) and np.all(fb2 == 0.0)
    assert np.all(ob2 == 0.0) and np.all(ab2 == 0.0)

    hf0 = (h0_new @ afw[0]).astype(ml_dtypes.bfloat16)          # conv-0 table

    offs = np.linspace(0.0, 5.0, NG).astype(np.float32)
    width = float(offs[1] - offs[0])
    coeff = -0.5 / (width * width)

    shared = {
        "fw1b": np.ascontiguousarray(
            fw1.transpose(1, 0, 2).reshape(NG, NC * NB)).astype(ml_dtypes.bfloat16),
        "fw2b": np.ascontiguousarray(
            fw2.transpose(1, 0, 2).reshape(NB, NC * NB)).astype(ml_dtypes.bfloat16),
        "afwf": np.ascontiguousarray(
            afw.transpose(1, 0, 2).reshape(NB, NC * NB)).astype(np.float32),
        "ow1w": np.ascontiguousarray(
            ow1.transpose(1, 0, 2).reshape(NB, NC * NB)).astype(np.float32),
        "ow2w": np.ascontiguousarray(
            ow2.transpose(1, 0, 2).reshape(NB, NC * NB)).astype(np.float32),
        "aw1w": aw1.astype(np.float32),                              # [NB,NH]
        "aw2w": aw2.astype(np.float32),                              # [NH,1]
        "fb1t": np.ascontiguousarray(fb1.T).astype(np.float32),      # [NB,NC]
        "ob1t": np.ascontiguousarray(ob1.T).astype(np.float32),
        "ab1t": ab1.reshape(NH, 1).astype(np.float32),
        "negmu": np.tile(-offs, 4).reshape(P, 1).astype(np.float32),
        "iotaf": np.tile(np.arange(P, dtype=np.float32), (P, 8)),
    }

    in_maps = []
    for c in range(C):
        m = dict(shared)
        osrc = pl.osrc_lin[c]
        odst = pl.odst_lin[c]
        xs = xyzf[osrc]
        xd = xyzf[odst]
        m["xsx"] = np.ascontiguousarray(xs[:, 0].reshape(P, F0))
        m["xsy"] = np.ascontiguousarray(xs[:, 1].reshape(P, F0))
        m["xsz"] = np.ascontiguousarray(xs[:, 2].reshape(P, F0))
        m["xdx"] = np.ascontiguousarray(xd[:, 0].reshape(P, F0))
        m["xdy"] = np.ascontiguousarray(xd[:, 1].reshape(P, F0))
        m["xdz"] = np.ascontiguousarray(xd[:, 2].reshape(P, F0))
        m["idx"] = np.ascontiguousarray(
            np.tile(pl.idx_lin[c].reshape(Ep // 16, 16).T, (8, 1)))
        m["dstrel"] = np.ascontiguousarray(
            pl.dstrel_lin[c].reshape(n_sub, P).T)
        m["h0t"] = np.ascontiguousarray(
            h0_new[c * SH:(c + 1) * SH].T)                          # [NB,SH]
        m["hftab0"] = hf0
        msk = np.zeros((K, P, NM), dtype=np.float32)
        mols = mol_new[c * SH:(c + 1) * SH].reshape(K, P)
        for mm in range(NM):
            msk[:, :, mm] = (mols == mm)
        m["mask"] = msk
        in_maps.append(m)
    return in_maps, coeff


# ----------------------------------------------------------------------------
# Device program
# ----------------------------------------------------------------------------

def _ap(tile_ap, extra_off, pattern):
    return bass.AP(tile_ap.tensor, tile_ap.offset + extra_off, pattern)


def _patch_act_tables():
    if getattr(bacc, "_act_tables_patched", False):
        return
    orig = bacc.get_activation_tables

    def patched(arch):
        t = dict(orig(arch))
        shared = {AF.Exp, AF.Ln, AF.Identity, AF.Copy, AF.Square}
        for name in t:
            if name != "natural_log_exp_and_others":
                t[name] = t[name] - shared
        return t

    bacc.get_activation_tables = patched
    bacc._act_tables_patched = True


def build_program(pl, NC, NM, coeff):
    _patch_act_tables()
    C, K, SH, Ep, n_sub = pl.n_cores, pl.K, pl.SH, pl.Ep, pl.n_sub
    F0 = Ep // P
    Q = Ep // 4
    NW = 8
    while Q % NW != 0 or (Q // NW) > 1024:
        NW *= 2
    Wg = Q // NW
    LMAX = max(ns for _, ns, _, _ in pl.spans)

    nc = bacc.Bacc("TRN2", target_bir_lowering=False, debug=False,
                   enable_asserts=False, num_devices=C, num_swdge_queues=4)

    def din(name, shape, dt=F32):
        return nc.dram_tensor(name, shape, dt, kind="ExternalInput").ap()

    xsx, xsy, xsz = din("xsx", [P, F0]), din("xsy", [P, F0]), din("xsz", [P, F0])
    xdx, xdy, xdz = din("xdx", [P, F0]), din("xdy", [P, F0]), din("xdz", [P, F0])
    idx_d = din("idx", [P, Ep // 16], I16)
    hftab0_d = din("hftab0", [pl.npad, NB], BF16)
    dstrel_d = din("dstrel", [P, n_sub])
    h0t_d = din("h0t", [NB, SH])
    mask_d = din("mask", [K, P, NM])
    fw1b_d = din("fw1b", [NG, NC * NB], BF16)
    fw2b_d = din("fw2b", [NB, NC * NB], BF16)
    afwf_d = din("afwf", [NB, NC * NB])
    ow1w_d = din("ow1w", [NB, NC * NB])
    ow2w_d = din("ow2w", [NB, NC * NB])
    aw1w_d = din("aw1w", [NB, NH])
    aw2w_d = din("aw2w", [NH, 1])
    fb1t_d = din("fb1t", [NB, NC])
    ob1t_d = din("ob1t", [NB, NC])
    ab1t_d = din("ab1t", [NH, 1])
    negmu_d = din("negmu", [P, 1])
    iotaf_d = din("iotaf", [P, 8 * P])

    ypart = nc.dram_tensor("ypart", [1, NM], F32, kind="ExternalOutput").ap()
    KDEBUG = bool(os.environ.get("KDEBUG"))
    if KDEBUG:
        aggdump = nc.dram_tensor("aggdump", [NB, SH], F32, kind="ExternalOutput").ap()
        hdump = nc.dram_tensor("hdump", [NB, SH], F32, kind="ExternalOutput").ap()

    with tile.TileContext(nc) as tc:
        with ExitStack() as ctx:
            dram = ctx.enter_context(tc.tile_pool(name="dram", bufs=1, space="DRAM"))
            res = ctx.enter_context(tc.tile_pool(name="res", bufs=1))
            sb = ctx.enter_context(tc.tile_pool(name="sb", bufs=3))
            gpool = ctx.enter_context(tc.tile_pool(name="gpool", bufs=6))
            p0 = ctx.enter_context(tc.tile_pool(name="p0", bufs=2))
            pps = ctx.enter_context(tc.tile_pool(name="pps", bufs=3, space="PSUM"))
            pp2 = pps  # ps1/ps2 share one 3-deep bank ring (same tag)
            ppagg = ctx.enter_context(tc.tile_pool(name="ppagg", bufs=2, space="PSUM"))
            ppsm = ctx.enter_context(tc.tile_pool(name="ppsm", bufs=2, space="PSUM"))
            ppem = ctx.enter_context(tc.tile_pool(name="ppem", bufs=1, space="PSUM"))

            # ---- DRAM scratch ----
            d_dram = dram.tile([P, F0], F32)
            g_dram = dram.tile([4, NG, Q], BF16)
            hf_dram = [dram.tile([pl.npad, NB], BF16, name=f"hftab{j}")
                       for j in range(2)]
            bounce = [dram.tile([SH, NB], BF16, name=f"bounce{j}")
                      for j in range(2)]

            # ---- resident SBUF ----
            h_my = res.tile([NB, SH], F32)
            agg_sb = res.tile([NB, SH], F32)
            idx_sb = res.tile([P, Ep // 16], I16)
            dstrel_sb = res.tile([P, n_sub], F32)
            iotaf_sb = res.tile([P, 8 * P], F32)
            mask_sb = res.tile([P, K * NM], F32)
            fw1b_sb = res.tile([NG, NC * NB], BF16)
            fw2b_sb = res.tile([NB, NC * NB], BF16)
            afwf_sb = res.tile([NB, NC * NB], F32)
            ow1_sb = res.tile([NB, NC * NB], F32)
            ow2_sb = res.tile([NB, NC * NB], F32)
            aw1_sb = res.tile([NB, NH], F32)
            aw2_sb = res.tile([NH, 1], F32)
            fb1_sb = res.tile([NB, NC], F32)
            ob1_sb = res.tile([NB, NC], F32)
            ab1_sb = res.tile([NH, 1], F32)
            negmu_sb = res.tile([P, 1], F32)
            epsb_sb = res.tile([P, 1], F32)
            nc.vector.memset(epsb_sb[:], EPS)
            halfb_sb = res.tile([P, 1], F32)
            nc.vector.memset(halfb_sb[:], 0.5)

            nc.sync.dma_start(idx_sb[:], idx_d[:])
            nc.sync.dma_start(h_my[:], h0t_d[:])
            nc.sync.dma_start(dstrel_sb[:], dstrel_d[:])
            nc.sync.dma_start(iotaf_sb[:], iotaf_d[:])
            nc.sync.dma_start(
                _ap(mask_sb[:], 0, [[K * NM, P], [NM, K], [1, NM]]),
                _ap(mask_d, 0, [[NM, P], [P * NM, K], [1, NM]]))
            for t_sb, t_d in [(fw1b_sb, fw1b_d), (fw2b_sb, fw2b_d),
                              (afwf_sb, afwf_d), (ow1_sb, ow1w_d),
                              (ow2_sb, ow2w_d), (aw1_sb, aw1w_d),
                              (aw2_sb, aw2w_d), (fb1_sb, fb1t_d),
                              (ob1_sb, ob1t_d), (ab1_sb, ab1t_d),
                              (negmu_sb, negmu_d)]:
                nc.sync.dma_start(t_sb[:], t_d[:])

            def hf_chunk(i, c):
                """Own-shard atom-filter features for conv i, chunk c ->
                bounce[i % 2] rows (atom-major bf16)."""
                hfps = ppsm.tile([P, NB], F32, tag="sm", name=f"hfps_{i}_{c}")
                nc.tensor.matmul(hfps[:], h_my[:, P * c:P * (c + 1)],
                                 afwf_sb[:, NB * i:NB * (i + 1)],
                                 start=True, stop=True, skip_group_check=True)
                hfsb = sb.tile([P, NB], BF16, tag="hfsb")
                nc.scalar.copy(hfsb[:], hfps[:])
                nc.sync.dma_start(
                    _ap(bounce[i % 2][:], c * P * NB, [[NB, P], [1, NB]]),
                    hfsb[:])

            # ================= phase 0: distances and gaussians ============
            # (the conv-0 gather table arrives precomputed as an input)
            cx = p0.tile([P, F0], F32, tag="ph0")
            cy = p0.tile([P, F0], F32, tag="ph0b")
            cz = p0.tile([P, F0], F32, tag="ph0c")
            tx = p0.tile([P, F0], F32, tag="ph0d")
            nc.sync.dma_start(cx[:], xsx[:])
            nc.sync.dma_start(tx[:], xdx[:])
            nc.vector.tensor_sub(cx[:], cx[:], tx[:])
            nc.vector.tensor_mul(cx[:], cx[:], cx[:])
            nc.sync.dma_start(cy[:], xsy[:])
            nc.sync.dma_start(tx[:], xdy[:])
            nc.vector.tensor_sub(cy[:], cy[:], tx[:])
            nc.vector.tensor_mul(cy[:], cy[:], cy[:])
            nc.sync.dma_start(cz[:], xsz[:])
            nc.sync.dma_start(tx[:], xdz[:])
            nc.vector.tensor_sub(cz[:], cz[:], tx[:])
            nc.vector.tensor_mul(cz[:], cz[:], cz[:])
            nc.vector.tensor_add(cx[:], cx[:], cy[:])
            nc.vector.tensor_add(cx[:], cx[:], cz[:])
            nc.scalar.activation(cy[:], cx[:], AF.Sqrt,
                                 bias=epsb_sb[:, 0:1], scale=1.0)
            nc.sync.dma_start(d_dram[:], cy[:])

            for w in range(NW):
                dbc = p0.tile([P, Wg], F32, tag="dbc")
                nc.sync.dma_start(
                    dbc[:], _ap(d_dram[:], w * Wg, [[Q, 4], [0, NG], [1, Wg]]))
                t1 = p0.tile([P, Wg], F32, tag="t1")
                nc.scalar.activation(t1[:], dbc[:], AF.Square,
                                     bias=negmu_sb[:, 0:1], scale=1.0)
                gt = p0.tile([P, Wg], BF16, tag="gt0")
                nc.scalar.activation(gt[:], t1[:], AF.Exp, bias=0.0, scale=coeff)
                nc.sync.dma_start(
                    _ap(g_dram[:], w * Wg, [[NG * Q, 4], [Q, NG], [1, Wg]]),
                    gt[:])

            # ================= conv layers =================================
            for i in range(NC):
                tbl = hftab0_d if i == 0 else hf_dram[i % 2][:]
                agg_open = {}

                def close_chunk(cki):
                    nc.scalar.copy(agg_sb[:, P * cki:P * (cki + 1)],
                                   agg_open[cki][:])
                    del agg_open[cki]
                    # atom update
                    ups = ppsm.tile([P, P], F32, tag="sm", name=f"ups_{i}_{cki}")
                    nc.tensor.matmul(ups[:], ow1_sb[:, NB * i:NB * (i + 1)],
                                     agg_sb[:, P * cki:P * (cki + 1)],
                                     start=True, stop=True, skip_group_check=True)
                    ue = sb.tile([P, P], F32, tag="ue")
                    nc.scalar.activation(ue[:], ups[:], AF.Exp,
                                         bias=ob1_sb[:, i:i + 1], scale=1.0)
                    usb = sb.tile([P, P], F32, tag="usb")
                    nc.scalar.activation(usb[:], ue[:], AF.Ln,
                                         bias=halfb_sb[:, 0:1], scale=0.5)
                    drps = ppsm.tile([P, P], F32, tag="sm", name=f"drps_{i}_{cki}")
                    nc.tensor.matmul(drps[:], ow2_sb[:, NB * i:NB * (i + 1)],
                                     usb[:], start=True, stop=True,
                                     skip_group_check=True)
                    nc.vector.tensor_add(h_my[:, P * cki:P * (cki + 1)],
                                         h_my[:, P * cki:P * (cki + 1)],
                                         drps[:])
                    if i < NC - 1:
                        hf_chunk(i + 1, cki)
                    else:
                        # fused readout for this chunk
                        r1ps = ppsm.tile([NH, P], F32, tag="sm",
                                         name=f"r1ps_{cki}")
                        nc.tensor.matmul(r1ps[:], aw1_sb[:],
                                         h_my[:, P * cki:P * (cki + 1)],
                                         start=True, stop=True,
                                         skip_group_check=True)
                        r1e = sb.tile([NH, P], F32, tag="r1e")
                        nc.scalar.activation(r1e[:], r1ps[:], AF.Exp,
                                             bias=ab1_sb[:, 0:1], scale=1.0)
                        r1sb = sb.tile([NH, P], F32, tag="r1sb")
                        nc.scalar.activation(r1sb[:], r1e[:], AF.Ln,
                                             bias=halfb_sb[0:NH, 0:1], scale=0.5)
                        yps = ppsm.tile([P, 1], F32, tag="sm", name=f"yps_{cki}")
                        nc.tensor.matmul(yps[:], r1sb[:], aw2_sb[:],
                                         start=True, stop=True,
                                         skip_group_check=True)
                        ysb = sb.tile([P, 1], F32, tag="ysb")
                        nc.scalar.copy(ysb[:], yps[:])
                        nc.tensor.matmul(em_ps[:], ysb[:],
                                         mask_sb[:, NM * cki:NM * (cki + 1)],
                                         start=(cki == 0), stop=(cki == K - 1),
                                         skip_group_check=True)
                        if cki == K - 1:
                            e_out = sb.tile([1, NM], F32, tag="eout")
                            nc.vector.tensor_copy(e_out[:], em_ps[:])
                            nc.sync.dma_start(ypart[:], e_out[:])

                if i == NC - 1:
                    em_ps = ppem.tile([1, NM], F32, name="emps")

                for (st0, nsx, half, cki) in pl.spans:
                    gbuf = gpool.tile([P, LMAX, NB], BF16, tag="gbuf")
                    if half == 0:
                        tbl_ap = _ap(tbl, 0, [[NB, pl.a_cap], [1, NB]])
                    else:
                        tbl_ap = _ap(tbl, pl.b_base * NB,
                                     [[NB, pl.npad - pl.b_base], [1, NB]])
                    cap = pl.span_cap[(cki, half)]
                    if os.environ.get("KNOGATHER"):
                        nc.vector.memset(gbuf[:, :nsx, :], 0.25)
                    else:
                        # the Q7 generator skips the trailing -1 indices;
                        # pre-zero the tail subtiles so the skipped slots
                        # hold finite data (NaN x Sm=0 would poison the
                        # scatter matmul), then gather the valid slots
                        if cap // P < nsx:
                            nc.vector.memset(gbuf[:, cap // P:nsx, :], 0.0)
                        # sub-calls of <=8 subtiles (<=1024 descriptors) keep
                        # each call within one 64-desc packet per SDMA engine
                        # so single-packet mode is legal; it removes the
                        # per-packet pickup overhead that dominates the drain
                        nsg = (cap + P - 1) // P
                        for s0 in range(0, nsg, 8):
                            s1 = min(s0 + 8, nsg)
                            reg = min(cap - s0 * P, (s1 - s0) * P)
                            nc.gpsimd.dma_gather(
                                gbuf[:, s0:s1, :], tbl_ap,
                                idx_sb[:, 8 * (st0 + s0):8 * (st0 + s1)],
                                reg, reg, NB,
                                single_packet=True)

                    if cki not in agg_open:
                        agg_open[cki] = ppagg.tile(
                            [P, P], F32, tag="aggps", name=f"aggps_{i}_{cki}")

                    for b0 in range(0, nsx, 4):
                        e0 = (st0 + b0) * P
                        q, col = e0 // Q, e0 % Q
                        gt2 = sb.tile([NG, 512], BF16, tag="gt2")
                        nc.sync.dma_start(gt2[:], g_dram[q, :, col:col + 512])
                        ps1 = pps.tile([P, 512], F32, tag="pblk", name=f"ps1_{i}_{st0}_{b0}")
                        nc.tensor.matmul(ps1[:],
                                         fw1b_sb[:, NB * i:NB * (i + 1)],
                                         gt2[:], start=True, stop=True)
                        nc.scalar.activation(ps1[:], ps1[:], AF.Exp,
                                             bias=fb1_sb[:, i:i + 1], scale=1.0)
                        x1 = sb.tile([P, 512], BF16, tag="x1")
                        nc.scalar.activation(x1[:], ps1[:], AF.Ln,
                                             bias=halfb_sb[:, 0:1], scale=0.5)
                        ps2 = pp2.tile([P, 512], F32, tag="pblk", name=f"ps2_{i}_{st0}_{b0}")
                        for s4 in range(4):
                            nc.tensor.matmul(
                                ps2[:, P * s4:P * (s4 + 1)],
                                x1[:, P * s4:P * (s4 + 1)],
                                fw2b_sb[:, NB * i:NB * (i + 1)],
                                start=True, stop=True, skip_group_check=True)
                        msg = sb.tile([P, 512], BF16, tag="msg")
                        hfg = _ap(gbuf[:], b0 * NB, [[LMAX * NB, P], [1, 512]])
                        nc.vector.tensor_tensor(msg[:], ps2[:], hfg, op=OP.mult)
                        Sm = sb.tile([P, 512], BF16, tag="Sm")
                        dr_ap = _ap(dstrel_sb[:], st0 + b0,
                                    [[n_sub, P], [1, 4], [0, P]])
                        nc.vector.tensor_tensor(
                            Sm[:], iotaf_sb[:, :512], dr_ap, op=OP.is_equal)
                        for s4 in range(4):
                            st = st0 + b0 + s4
                            first = (st == pl.chunk_first[cki])
                            last = (st == pl.chunk_last[cki])
                            nc.tensor.matmul(
                                agg_open[cki][:],
                                msg[:, P * s4:P * (s4 + 1)],
                                Sm[:, P * s4:P * (s4 + 1)],
                                start=first, stop=last,
                                skip_group_check=True)
                            if last:
                                close_chunk(cki)

                if i < NC - 1:
                    nc.gpsimd.collective_compute(
                        "AllGather", OP.bypass,
                        replica_groups=[list(range(C))],
                        ins=[bounce[(i + 1) % 2].opt()],
                        outs=[hf_dram[(i + 1) % 2].opt()])
                if KDEBUG and i == 0:
                    nc.sync.dma_start(aggdump[:], agg_sb[:])
                    nc.sync.dma_start(hdump[:], h_my[:])

    # Spread gather descriptor-generation across the 4 SWDGE queues,
    # consistent with the DMASW semaphore lane Tile assigned.
    import concourse.tile_sem_assignment as tsa
    sw_procs = {tsa.PROC_NAME_TO_IDX[f"DMASW{k}"]: k for k in range(8)}
    locked0 = set()
    gathers = []
    for b in nc.main_func.blocks:
        for inst in b.instructions:
            proc = getattr(inst, "bass_scheduled_proc", None)
            if proc in sw_procs:
                if isinstance(inst, mybir.InstDMAGatherAnt):
                    gathers.append((inst, sw_procs[proc]))
                else:
                    locked0.add(sw_procs[proc])
    for inst, lane in gathers:
        inst.queue_num = 0 if lane in locked0 else lane % 4

    nc.compile()
    return nc


# ----------------------------------------------------------------------------
# Entry point
# ----------------------------------------------------------------------------

_CACHE = {}


def _get_program(pl, NC, NM, coeff):
    key = (pl.n_atoms, pl.n_edges, pl.Ep, pl.K, NC, NM, round(coeff, 9))
    if key not in _CACHE:
        _CACHE[key] = build_program(pl, NC, NM, coeff)
    return _CACHE[key]


def kernel(r, xyz, a, n_per, embed, fw1, fb1, fw2, fb2, afw, afb,
           ow1, ob1, ow2, ob2, aw1, ab1, aw2, ab2, trace=False):
    r = np.asarray(r)
    xyz = np.asarray(xyz, dtype=np.float32)
    a = np.asarray(a)
    weights = dict(fw1=np.asarray(fw1), fb1=np.asarray(fb1),
                   fw2=np.asarray(fw2), fb2=np.asarray(fb2),
                   afw=np.asarray(afw), afb=np.asarray(afb),
                   ow1=np.asarray(ow1), ob1=np.asarray(ob1),
                   ow2=np.asarray(ow2), ob2=np.asarray(ob2),
                   aw1=np.asarray(aw1), ab1=np.asarray(ab1),
                   aw2=np.asarray(aw2), ab2=np.asarray(ab2))
    pl = make_plan(r, xyz, a, int(n_per), n_cores=8)
    in_maps, coeff = make_inputs(pl, r, xyz, a, np.asarray(embed), weights)
    NC = weights["fw1"].shape[0]
    nc = _get_program(pl, NC, pl.n_mol, coeff)
    res = bass_utils.run_bass_kernel_spmd(
        nc, in_maps, core_ids=list(range(pl.n_cores)), trace=trace)
    out = np.zeros(pl.n_mol, dtype=np.float64)
    for k in range(pl.n_cores):
        out += res.results[k]["ypart"][0].astype(np.float64)
    kernel._last_results = res
    return out.astype(np.float32)
